# revision 2
# baseline (speedup 1.0000x reference)
"""Causal self-attention (B=4, T=2048, C=1024, 16 heads) on 8 trn2 NeuronCores.

Sharding: batch x head-group hybrid. Core c handles batch c//2 and head
group c%2 (8 of 16 heads). Each core computes qkv projection for its
head group over its batch's tokens, runs causal flash-style attention
for its 8 heads, and produces a partial c_proj output (contraction over
its 512 of the 1024 y-channels). Host sums the two partials per batch
and adds b_proj.

On-chip layout (per core):
  x^T  [C, T]  built via PE transposes (PE contracts over partitions).
  Q^T, K^T [j, T] (j = head-major 64-dim blocks, bf16) via W_qk^T @ x^T.
  V'   [T, 65] per head (fp32, col 64 = ones so P@V' also yields softmax
       denominators as row 64 of the PSUM output).
  S^T  tiles [k_tok 128, q 512] = K^T_tile.T @ Q^T_chunk, two heads packed
       in one PE pass via row-groups (contract dim is only 64).
  P    = exp(0.125 * S^T) via ScalarE, causal diag blocks masked by a
       triu multiply on VectorE; fully-masked columns never computed.
  O'   [65, q] accumulated over k tiles: V'.T @ P.
  y    [128, T] per head-pair; head B's O' rows are moved into partitions
       64..128 with a SBUF->SBUF DMA (engines cannot shift partitions).
  out  partial [T, C] = y_pair.T @ W_proj_local rows, accumulated in PSUM.
"""

from contextlib import ExitStack

import numpy as np

import concourse.bass as bass
import concourse.mybir as mybir
import concourse.tile as tile
from concourse import bacc
from concourse.bass_utils import run_bass_kernel_spmd
from concourse.masks import make_identity, make_upper_triangular

F32 = mybir.dt.float32
BF16 = mybir.dt.bfloat16

T = 2048
C = 1024
NH_LOC = 8          # heads per core
HD = 64
J = NH_LOC * HD     # 512 local q/k/v channels
N_CORES = 8
QC = 4              # q chunks of 512
TOK_TILES = 16      # token tiles of 128
C_TILES = 8         # contraction tiles of 128 over C
PAIRS = 4           # head pairs per core

QK_DT = BF16        # Q^T/K^T storage dtype (SBUF budget)


def build_nc():
    nc = bacc.Bacc("TRN2", target_bir_lowering=False, debug=False)

    x_d = nc.dram_tensor("x", [T, C], F32, kind="ExternalInput")
    wqk_d = nc.dram_tensor("wqk", [C, 2 * J], F32, kind="ExternalInput")
    wv_d = nc.dram_tensor("wv", [C, J], F32, kind="ExternalInput")
    bqk_d = nc.dram_tensor("bqk", [2 * J], F32, kind="ExternalInput")
    bv_d = nc.dram_tensor("bv", [J], F32, kind="ExternalInput")
    wp_d = nc.dram_tensor("wp", [J, C], F32, kind="ExternalInput")
    out_d = nc.dram_tensor("out", [T, C], F32, kind="ExternalOutput")

    with tile.TileContext(nc) as tc, ExitStack() as ctx:
        const = ctx.enter_context(tc.tile_pool(name="const", bufs=1))
        wpool = ctx.enter_context(tc.tile_pool(name="w", bufs=1))
        qkv = ctx.enter_context(tc.tile_pool(name="qkv", bufs=1))
        ypool = ctx.enter_context(tc.tile_pool(name="y", bufs=1))
        psum = ctx.enter_context(tc.tile_pool(name="psum", bufs=1, space="PSUM"))

        # ---- constants ----
        ident = const.tile([128, 128], F32)
        make_identity(nc, ident)
        triu = const.tile([128, 128], F32)
        make_upper_triangular(nc, triu, val=1.0, diag=True)
        ones_row = const.tile([1, 128], F32)
        nc.vector.memset(ones_row, 1.0)
        # selab[p, f] = 1 iff f in [64p, 64p+64): head-select for the
        # reciprocal broadcast matmul. Partition-1 memsets are illegal, so
        # carve it with two affine_selects (iota = base + cm*p + step*f).
        selab = const.tile([2, 128], F32)
        nc.gpsimd.memset(selab, 1.0)
        nc.gpsimd.affine_select(
            out=selab, in_=selab, compare_op=mybir.AluOpType.is_ge,
            fill=0.0, base=0, pattern=[[1, 128]], channel_multiplier=-64)
        nc.gpsimd.affine_select(
            out=selab, in_=selab, compare_op=mybir.AluOpType.is_ge,
            fill=0.0, base=63, pattern=[[-1, 128]], channel_multiplier=64)
        bqk_sb = const.tile([128, 8], F32)
        nc.sync.dma_start(bqk_sb, bqk_d[:].rearrange("(t p) -> p t", p=128))
        bv_sb = const.tile([1, J], F32)
        nc.sync.dma_start(bv_sb, bv_d[:].rearrange("(a n) -> a n", a=1))

        # ---- resident weights ----
        wqk_sb = []
        for ct in range(C_TILES):
            w = wpool.tile([128, 2 * J], F32, name=f"wqk{ct}")
            nc.sync.dma_start(w, wqk_d[ct * 128:(ct + 1) * 128, :])
            wqk_sb.append(w)
        wv_sb = []
        for ct in range(C_TILES):
            w = wpool.tile([128, J], F32, name=f"wv{ct}")
            nc.sync.dma_start(w, wv_d[ct * 128:(ct + 1) * 128, :])
            wv_sb.append(w)

        # ---- persistent activations ----
        qt_sb = [qkv.tile([128, T], QK_DT, name=f"qt{p}") for p in range(PAIRS)]
        kt_sb = [qkv.tile([128, T], QK_DT, name=f"kt{p}") for p in range(PAIRS)]
        v_sb = [qkv.tile([128, TOK_TILES, 65], F32, name=f"v{h}")
                for h in range(NH_LOC)]
        for h in range(NH_LOC):
            nc.vector.memset(v_sb[h][:, :, 64:65], 1.0)
        y_sb = [ypool.tile([128, T], F32, name=f"y{p}") for p in range(PAIRS)]

        # ================= phase A: x^T, qkv projections =================
        with tc.tile_pool(name="pa", bufs=1) as pa, \
             tc.tile_pool(name="psa", bufs=1, space="PSUM") as psa:
            for qc in range(QC):
                xT = pa.tile([128, C_TILES, 512], F32, tag="xT", bufs=2)
                for tt in range(4):
                    xa = pa.tile([128, C], F32, tag="x", bufs=3)
                    r0 = qc * 512 + tt * 128
                    nc.sync.dma_start(xa, x_d[r0:r0 + 128, :])
                    for ct in range(C_TILES):
                        tp = psa.tile([128, 128], F32, tag="tp", bufs=2)
                        nc.tensor.transpose(tp, xa[:, ct * 128:(ct + 1) * 128], ident)
                        nc.vector.tensor_copy(
                            xT[:, ct, tt * 128:(tt + 1) * 128], tp)
                # QK projection for this token chunk -> Q^T/K^T columns
                for jt in range(8):
                    pm = psa.tile([128, 512], F32, tag="mm", bufs=4)
                    for ct in range(C_TILES):
                        nc.tensor.matmul(
                            pm,
                            wqk_sb[ct][:, jt * 128:(jt + 1) * 128],
                            xT[:, ct, :],
                            start=(ct == 0), stop=(ct == C_TILES - 1))
                    dst = qt_sb[jt] if jt < 4 else kt_sb[jt - 4]
                    nc.scalar.activation(
                        dst[:, qc * 512:(qc + 1) * 512], pm,
                        mybir.ActivationFunctionType.Identity,
                        bias=bqk_sb[:, jt:jt + 1])
                # V projection for this token chunk (natural layout + bias)
                for tt in range(4):
                    tta = qc * 4 + tt
                    pv = psa.tile([128, J], F32, tag="mm", bufs=4)
                    for ct in range(C_TILES):
                        nc.tensor.matmul(
                            pv,
                            xT[:, ct, tt * 128:(tt + 1) * 128],
                            wv_sb[ct],
                            start=(ct == 0), stop=False)
                    nc.tensor.matmul(pv, ones_row, bv_sb,
                                     start=False, stop=True)
                    for h in range(NH_LOC):
                        nc.vector.tensor_copy(
                            v_sb[h][:, tta, 0:64], pv[:, h * 64:(h + 1) * 64])

        # ================= phase B/C: attention per head pair =============
        with tc.tile_pool(name="pb", bufs=1) as pb, \
             tc.tile_pool(name="psb", bufs=1, space="PSUM") as psb:
            for p in range(PAIRS):
                for qc in range(QC):
                    q0 = qc * 512
                    o_a = psb.tile([65, 512], F32, tag="O", bufs=3, name="o_a")
                    o_b = psb.tile([65, 512], F32, tag="O", bufs=3, name="o_b")
                    n_kt = 4 * (qc + 1)
                    for kt in range(n_kt):
                        off = max(0, kt * 128 - q0)
                        w = 512 - off
                        s_a = psb.tile([128, 512], F32, tag="S", bufs=3,
                                       name="s_a")
                        s_b = psb.tile([128, 512], F32, tag="S", bufs=3,
                                       name="s_b")
                        for half, s_ps in ((0, s_a), (1, s_b)):
                            r0, r1 = half * 64, half * 64 + 64
                            nc.tensor.matmul(
                                s_ps[:, off:512],
                                kt_sb[p][r0:r1, kt * 128:(kt + 1) * 128],
                                qt_sb[p][r0:r1, q0 + off:q0 + 512],
                                start=True, stop=True)
                        p_a = pb.tile([128, 512], F32, tag="P", bufs=4,
                                      name="p_a")
                        p_b = pb.tile([128, 512], F32, tag="P", bufs=4,
                                      name="p_b")
                        diag = kt * 128 >= q0
                        for s_ps, p_sb in ((s_a, p_a), (s_b, p_b)):
                            nc.scalar.activation(
                                p_sb[:, off:512], s_ps[:, off:512],
                                mybir.ActivationFunctionType.Exp, scale=0.125)
                            if diag:
                                nc.vector.tensor_mul(
                                    p_sb[:, off:off + 128],
                                    p_sb[:, off:off + 128], triu)
                        first, last = (kt == 0), (kt == n_kt - 1)
                        nc.tensor.matmul(o_a[:, off:512],
                                         v_sb[2 * p][:, kt, :],
                                         p_a[:, off:512],
                                         start=first, stop=last)
                        nc.tensor.matmul(o_b[:, off:512],
                                         v_sb[2 * p + 1][:, kt, :],
                                         p_b[:, off:512],
                                         start=first, stop=last)
                    # head A: rows already aligned; stage only the sum row
                    nc.vector.tensor_copy(y_sb[p][0:64, q0:q0 + 512],
                                          o_a[0:64, :])
                    stg_a = pb.tile([65, 512], F32, tag="stg", bufs=2,
                                    name="stg_a")
                    nc.scalar.copy(stg_a[64:65, :], o_a[64:65, :])
                    # head B: stage all rows, DMA shifts partitions 0..63->64..127
                    stg_b = pb.tile([65, 512], F32, tag="stg", bufs=2,
                                    name="stg_b")
                    nc.vector.tensor_copy(stg_b, o_b)
                    sums = pb.tile([2, 512], F32, tag="sums", bufs=2)
                    nc.sync.dma_start(sums[0:1, :], stg_a[64:65, :])
                    nc.sync.dma_start(y_sb[p][64:128, q0:q0 + 512],
                                      stg_b[0:64, :])
                    nc.sync.dma_start(sums[1:2, :], stg_b[64:65, :])
                    rec = pb.tile([2, 512], F32, tag="rec", bufs=2)
                    nc.vector.reciprocal_approx_fast(rec, sums)
                    bc = psb.tile([128, 512], F32, tag="bc", bufs=2)
                    nc.tensor.matmul(bc, selab, rec, start=True, stop=True)
                    nc.vector.tensor_mul(y_sb[p][:, q0:q0 + 512],
                                         y_sb[p][:, q0:q0 + 512], bc)

        # ================= phase D: output projection =====================
        with tc.tile_pool(name="pd", bufs=1) as pd, \
             tc.tile_pool(name="psd", bufs=1, space="PSUM") as psd:
            wp_sb = []
            for p in range(PAIRS):
                w = pd.tile([128, C], F32, tag="wp", bufs=4, name=f"wp{p}")
                nc.sync.dma_start(w, wp_d[p * 128:(p + 1) * 128, :])
                wp_sb.append(w)
            for tt in range(TOK_TILES):
                for oc in range(2):
                    po = psd.tile([128, 512], F32, tag="mm", bufs=4)
                    for p in range(PAIRS):
                        nc.tensor.matmul(
                            po,
                            y_sb[p][:, tt * 128:(tt + 1) * 128],
                            wp_sb[p][:, oc * 512:(oc + 1) * 512],
                            start=(p == 0), stop=(p == PAIRS - 1))
                    ob = pd.tile([128, 512], F32, tag="ob", bufs=4)
                    nc.scalar.copy(ob, po)
                    nc.sync.dma_start(
                        out_d[tt * 128:(tt + 1) * 128,
                              oc * 512:(oc + 1) * 512], ob)

    nc.compile()
    return nc


_NC_CACHE = {}


def _get_nc():
    if "nc" not in _NC_CACHE:
        _NC_CACHE["nc"] = build_nc()
    return _NC_CACHE["nc"]


def shard_inputs(x, W_attn, b_attn, W_proj):
    """Per-core input maps. Core c: batch c//2, head group c%2."""
    x = np.ascontiguousarray(np.asarray(x, dtype=np.float32))
    W_attn = np.asarray(W_attn, dtype=np.float32)
    b_attn = np.asarray(b_attn, dtype=np.float32)
    W_proj = np.asarray(W_proj, dtype=np.float32)
    in_maps = []
    for c in range(N_CORES):
        b, hg = c // 2, c % 2
        qs, ks, vs = hg * J, C + hg * J, 2 * C + hg * J
        wqk = np.ascontiguousarray(
            np.concatenate([W_attn[:, qs:qs + J], W_attn[:, ks:ks + J]], axis=1))
        wv = np.ascontiguousarray(W_attn[:, vs:vs + J])
        bqk = np.ascontiguousarray(
            np.concatenate([b_attn[qs:qs + J], b_attn[ks:ks + J]]))
        bv = np.ascontiguousarray(b_attn[vs:vs + J])
        wp = np.ascontiguousarray(W_proj[hg * J:(hg + 1) * J, :])
        in_maps.append({
            "x": np.ascontiguousarray(x[b]),
            "wqk": wqk, "wv": wv, "bqk": bqk, "bv": bv, "wp": wp,
        })
    return in_maps


def kernel(x, W_attn, b_attn, W_proj, b_proj):
    nc = _get_nc()
    in_maps = shard_inputs(x, W_attn, b_attn, W_proj)
    res = run_bass_kernel_spmd(nc, in_maps, list(range(N_CORES)))
    b_proj = np.asarray(b_proj, dtype=np.float32)
    outs = []
    for b in range(4):
        partial = res.results[2 * b]["out"] + res.results[2 * b + 1]["out"]
        outs.append(partial + b_proj[None, :])
    return np.stack(outs, axis=0)


# revision 32
# speedup vs baseline: 2.5457x; 2.5457x over previous
"""Causal self-attention (B=4, T=2048, C=1024, 16 heads) on 8 trn2 NeuronCores.

Sharding: batch x head-group hybrid. Core c handles batch c//2 and head
group c%2 (8 of 16 heads). Each core computes the qkv projection for its
head group over its batch's tokens, runs causal attention for its 8
heads, and produces a partial c_proj output (contraction over its 512 of
the 1024 y channels). Host sums the two partials per batch, adds b_proj.

PE contracts over the partition dim, so x is laid out transposed (x^T)
once via PE transposes; after that every matmul chains without further
transposes:
  x^T [c, tok]        PE transpose (fp32 in, bf16 out)
  Q^T, K^T [j, tok]   = W_qk^T x^T   (j head-major, bf16)
  V' [tok, 65]        = x W_v        (bf16; col 64 = ones so that P@V'
                                      also emits softmax denominators)
  S^T [k_tok, q]      = K^T_tile.T Q^T  two heads packed per PE pass via
                        row groups (contract dim is 64); both heads land
                        in one 2-bank PSUM tile so a single ScalarE exp
                        covers them.
  P = exp(S^T/8)      bf16; causal diagonal blocks masked by a triu
                        multiply; fully-masked columns never computed.
  O' [65, q]          = V'.T P accumulated over k tiles.
  y [128, tok]        per head pair, bf16. Head B's O' rows are shifted
                        into partitions 64..128 by a SBUF->SBUF DMA
                        (compute engines cannot cross partitions).
  out partial [tok, C] = y_pair.T W_proj_rows accumulated over pairs.

Scheduling notes:
  - All matmul operands are bf16 (fp32 matmuls cost 4 cyc/row, bf16 1).
  - Phase A transposes for chunk qc+1 are emitted before the projection
    matmuls of chunk qc so PE is never starved by the PSUM->SBUF copies.
  - Attention loops qc-outer so the output projection of chunk qc can be
    emitted (and run) while attention for qc+1 proceeds.
  - Each group's normalization tail (copies -> SBUF DMAs -> reciprocal ->
    GpSimd partition-broadcast -> multiply) has no PE instructions; only
    the final multiply is deferred into the next group so the VectorE
    queue is not blocked behind the chain.
Measured end-to-end relative error vs the fp32 reference: ~2e-3.
"""

from contextlib import ExitStack

import numpy as np
import ml_dtypes

import concourse.bass as bass
import concourse.mybir as mybir
import concourse.tile as tile
from concourse import bacc
from concourse.bass_utils import run_bass_kernel_spmd
from concourse.masks import make_identity

F32 = mybir.dt.float32
BF16 = mybir.dt.bfloat16

T = 2048
C = 1024
NH_LOC = 8          # heads per core
HD = 64
J = NH_LOC * HD     # 512 local q/k/v channels
N_CORES = 8
QC = 4              # q chunks of 512
TOK_TILES = 16      # token tiles of 128
C_TILES = 8         # contraction tiles of 128 over C
PAIRS = 4           # head pairs per core


def build_nc(debug_taps=False):
    nc = bacc.Bacc("TRN2", target_bir_lowering=False, debug=False)
    dbg = {}
    if debug_taps:
        dbg["y"] = nc.dram_tensor("dbg_y", [PAIRS * 128, T], BF16,
                                  kind="ExternalOutput")
        dbg["sums"] = nc.dram_tensor("dbg_sums", [PAIRS * QC * 2, 512], F32,
                                     kind="ExternalOutput")
        dbg["qt"] = nc.dram_tensor("dbg_qt", [PAIRS * 128, T], BF16,
                                   kind="ExternalOutput")
        dbg["kt"] = nc.dram_tensor("dbg_kt", [PAIRS * 128, T], BF16,
                                   kind="ExternalOutput")
        dbg["v"] = nc.dram_tensor("dbg_v", [NH_LOC * 128, TOK_TILES * 65],
                                  BF16, kind="ExternalOutput")

    x_d = nc.dram_tensor("x", [T, C], F32, kind="ExternalInput")
    wqk_d = nc.dram_tensor("wqk", [C, 2 * J], BF16, kind="ExternalInput")
    wv_d = nc.dram_tensor("wv", [C, J], BF16, kind="ExternalInput")
    bqk_d = nc.dram_tensor("bqk", [2 * J], F32, kind="ExternalInput")
    bv_d = nc.dram_tensor("bv", [J], BF16, kind="ExternalInput")
    wp_d = nc.dram_tensor("wp", [J, C], BF16, kind="ExternalInput")
    out_d = nc.dram_tensor("out", [T, C], F32, kind="ExternalOutput")

    with tile.TileContext(nc) as tc, ExitStack() as ctx:
        const = ctx.enter_context(tc.tile_pool(name="const", bufs=1))
        wpool = ctx.enter_context(tc.tile_pool(name="w", bufs=1))
        qkv = ctx.enter_context(tc.tile_pool(name="qkv", bufs=1))
        ypool = ctx.enter_context(tc.tile_pool(name="y", bufs=1))
        wk = ctx.enter_context(tc.tile_pool(name="wk", bufs=1))

        # ---- constants ----
        ident = const.tile([128, 128], F32)
        make_identity(nc, ident)
        # triu2[p, c, f] = 1 iff f >= p, duplicated over c: masks the causal
        # diagonal 128-block of both heads' P in one tensor_tensor op.
        triu2 = const.tile([128, 2, 128], BF16)
        nc.gpsimd.memset(triu2, 0.0)
        nc.gpsimd.affine_select(
            out=triu2, in_=triu2, compare_op=mybir.AluOpType.is_gt,
            fill=1.0, base=0, pattern=[[0, 2], [-1, 128]],
            channel_multiplier=1)
        ones_row = const.tile([1, 128], BF16)
        nc.vector.memset(ones_row, 1.0)
        # selab[p, f] = 1 iff f in [64p, 64p+64): head selector for the
        # reciprocal broadcast matmul (partition-1 memsets are illegal).
        selab = const.tile([2, 128], F32)
        nc.gpsimd.memset(selab, 1.0)
        nc.gpsimd.affine_select(
            out=selab, in_=selab, compare_op=mybir.AluOpType.is_ge,
            fill=0.0, base=0, pattern=[[1, 128]], channel_multiplier=-64)
        nc.gpsimd.affine_select(
            out=selab, in_=selab, compare_op=mybir.AluOpType.is_ge,
            fill=0.0, base=63, pattern=[[-1, 128]], channel_multiplier=64)
        bqk_sb = const.tile([128, 8], F32)
        nc.sync.dma_start(bqk_sb, bqk_d[:].rearrange("(t p) -> p t", p=128))
        bv_sb = const.tile([1, J], BF16)
        nc.sync.dma_start(bv_sb, bv_d[:].rearrange("(a n) -> a n", a=1))

        # ---- resident weights (bf16) ----
        wqk_sb = []
        for ct in range(C_TILES):
            w = wpool.tile([128, 2 * J], BF16, name=f"wqk{ct}")
            nc.sync.dma_start(w, wqk_d[ct * 128:(ct + 1) * 128, :])
            wqk_sb.append(w)
        wv_sb = []
        for ct in range(C_TILES):
            w = wpool.tile([128, J], BF16, name=f"wv{ct}")
            nc.sync.dma_start(w, wv_d[ct * 128:(ct + 1) * 128, :])
            wv_sb.append(w)
        wp_sb = []
        for p in range(PAIRS):
            w = wpool.tile([128, C], BF16, name=f"wp{p}")
            nc.sync.dma_start(w, wp_d[p * 128:(p + 1) * 128, :])
            wp_sb.append(w)

        # ---- persistent activations ----
        qt_sb = [qkv.tile([128, T], BF16, name=f"qt{p}") for p in range(PAIRS)]
        kt_sb = [qkv.tile([128, T], BF16, name=f"kt{p}") for p in range(PAIRS)]
        v_sb = [qkv.tile([128, TOK_TILES, 65], BF16, name=f"v{h}")
                for h in range(NH_LOC)]
        for h in range(NH_LOC):
            nc.vector.memset(v_sb[h][:, :, 64:65], 1.0)
        y_sb = [ypool.tile([128, T], BF16, name=f"y{p}") for p in range(PAIRS)]

        # ================= phase A: x^T, qkv projections =================
        with tc.tile_pool(name="psa", bufs=1, space="PSUM") as psa:

            def emit_transposes(qc):
                xT = wk.tile([128, C_TILES, 512], BF16, tag="xT", bufs=2)
                for tt in range(4):
                    xa = wk.tile([128, C], F32, tag="x", bufs=8)
                    r0 = qc * 512 + tt * 128
                    nc.sync.dma_start(xa, x_d[r0:r0 + 128, :])
                    for ct in range(C_TILES):
                        tp = psa.tile([128, 128], F32, tag="tp", bufs=4)
                        nc.tensor.transpose(
                            tp, xa[:, ct * 128:(ct + 1) * 128], ident)
                        nc.vector.tensor_copy(
                            xT[:, ct, tt * 128:(tt + 1) * 128], tp)
                return xT

            def emit_projs(qc, xT):
                # QK projection for this token chunk -> Q^T/K^T columns
                for jt in range(8):
                    pm = psa.tile([128, 512], F32, tag="mm", bufs=4)
                    for ct in range(C_TILES):
                        nc.tensor.matmul(
                            pm,
                            wqk_sb[ct][:, jt * 128:(jt + 1) * 128],
                            xT[:, ct, :],
                            start=(ct == 0), stop=(ct == C_TILES - 1))
                    dst = qt_sb[jt] if jt < 4 else kt_sb[jt - 4]
                    nc.scalar.activation(
                        dst[:, qc * 512:(qc + 1) * 512], pm,
                        mybir.ActivationFunctionType.Identity,
                        bias=bqk_sb[:, jt:jt + 1])
                # V projection for this token chunk (natural layout + bias)
                for tt in range(4):
                    tta = qc * 4 + tt
                    pv = psa.tile([128, J], F32, tag="mm", bufs=4)
                    for ct in range(C_TILES):
                        nc.tensor.matmul(
                            pv,
                            xT[:, ct, tt * 128:(tt + 1) * 128],
                            wv_sb[ct],
                            start=(ct == 0), stop=False)
                    nc.tensor.matmul(pv, ones_row, bv_sb,
                                     start=False, stop=True)
                    for h in range(NH_LOC):
                        # ScalarE is idle in phase A; keep VectorE for x^T
                        nc.scalar.copy(
                            v_sb[h][:, tta, 0:64], pv[:, h * 64:(h + 1) * 64])

            xTs = {0: emit_transposes(0)}
            for qc in range(QC):
                if qc + 1 < QC:
                    xTs[qc + 1] = emit_transposes(qc + 1)
                emit_projs(qc, xTs.pop(qc))

        # ============ phase B/D: attention + overlapped projection ========
        with tc.tile_pool(name="psb", bufs=1, space="PSUM") as psb:
            # Each group's normalization tail is a long cross-engine chain:
            # copies -> SBUF DMAs -> reciprocal -> partition-broadcast ->
            # multiply. Nothing in it runs on PE; the final multiply is
            # deferred into the next group so VectorE isn't blocked behind
            # the chain when the next group's mask multiplies come up.
            pending = []

            def flush_one():
                if pending:
                    pending.pop(0)()

            def flush_pending():
                while pending:
                    pending.pop(0)()

            def emit_tail(p, q0, o_a, o_b):
                # head A rows land aligned; stage sums + head B rows
                nc.vector.tensor_copy(y_sb[p][0:64, q0:q0 + 512],
                                      o_a[0:64, :])
                stg_b = wk.tile([64, 512], BF16, tag="stgb", bufs=2)
                nc.vector.tensor_copy(stg_b, o_b[0:64, :])
                stg_s = wk.tile([65, 1024], F32, tag="stgs", bufs=2)
                nc.scalar.copy(stg_s[64:65, 0:512], o_a[64:65, :])
                nc.scalar.copy(stg_s[64:65, 512:1024], o_b[64:65, :])
                sums = wk.tile([2, 512], F32, tag="sums", bufs=2)
                nc.sync.dma_start(sums[0:1, :], stg_s[64:65, 0:512])
                nc.sync.dma_start(sums[1:2, :], stg_s[64:65, 512:1024])
                nc.sync.dma_start(y_sb[p][64:128, q0:q0 + 512], stg_b)
                if debug_taps:
                    row = (p * QC + q0 // 512) * 2
                    nc.sync.dma_start(dbg["sums"][row:row + 2, :], sums)
                rec = wk.tile([2, 512], F32, tag="rec", bufs=2)
                nc.vector.reciprocal_approx_fast(rec, sums)

                def fin():
                    bc = psb.tile([128, 512], F32, tag="bcpo", bufs=1, name="bc")
                    nc.tensor.matmul(bc, selab, rec, start=True, stop=True)
                    nc.vector.tensor_mul(y_sb[p][:, q0:q0 + 512],
                                         y_sb[p][:, q0:q0 + 512], bc)
                pending.append(fin)

            def make_proj_piece(tt, oc):
                # one output-projection accumulation for token tile tt
                # (needs all 4 pairs' y columns for tt normalized)
                def proj():
                    po = psb.tile([128, 512], F32, tag="bcpo", bufs=1,
                                  name="po")
                    for p in range(PAIRS):
                        nc.tensor.matmul(
                            po,
                            y_sb[p][:, tt * 128:(tt + 1) * 128],
                            wp_sb[p][:, oc * 512:(oc + 1) * 512],
                            start=(p == 0), stop=(p == PAIRS - 1))
                    ob = wk.tile([128, 512], F32, tag="ob", bufs=4)
                    nc.vector.tensor_copy(ob, po)
                    nc.sync.dma_start(
                        out_d[tt * 128:(tt + 1) * 128,
                              oc * 512:(oc + 1) * 512], ob)
                return proj

            for qc in range(QC):
                q0 = qc * 512
                n_kt = 4 * (qc + 1)
                for p in range(PAIRS):
                    o_a = psb.tile([65, 512], F32, tag="O", bufs=3, name="o_a")
                    o_b = psb.tile([65, 512], F32, tag="O", bufs=3, name="o_b")
                    # software pipeline: emit S(kt) one step ahead of its
                    # exp/mask/PV consumers so PE never waits for ScalarE.
                    staged = {}

                    def emit_s(kt):
                        off = max(0, kt * 128 - q0)
                        # S for both heads in one 2-bank psum tile so one
                        # ScalarE exp covers both
                        s_ab = psb.tile([128, 1024], F32, tag="S", bufs=2,
                                        name="s_ab")
                        for half in range(2):
                            r0, r1 = half * 64, half * 64 + 64
                            nc.tensor.matmul(
                                s_ab[:, half * 512 + off:half * 512 + 512],
                                kt_sb[p][r0:r1, kt * 128:(kt + 1) * 128],
                                qt_sb[p][r0:r1, q0 + off:q0 + 512],
                                start=True, stop=True)
                        staged[kt] = (s_ab, off)

                    def emit_consume(kt):
                        s_ab, off = staged.pop(kt)
                        p_ab = wk.tile([128, 1024], BF16, tag="P", bufs=4,
                                       name="p_ab")
                        s3 = s_ab.rearrange("p (c w) -> p c w", c=2)
                        p3 = p_ab.rearrange("p (c w) -> p c w", c=2)
                        nc.scalar.activation(
                            p3[:, :, off:512], s3[:, :, off:512],
                            mybir.ActivationFunctionType.Exp, scale=0.125)
                        if kt * 128 >= q0:  # causal diagonal block
                            nc.vector.tensor_mul(
                                p3[:, :, off:off + 128],
                                p3[:, :, off:off + 128], triu2)
                        first, last = (kt == 0), (kt == n_kt - 1)
                        nc.tensor.matmul(o_a[:, off:512],
                                         v_sb[2 * p][:, kt, :],
                                         p_ab[:, off:512],
                                         start=first, stop=last)
                        nc.tensor.matmul(o_b[:, off:512],
                                         v_sb[2 * p + 1][:, kt, :],
                                         p_ab[:, 512 + off:1024],
                                         start=first, stop=last)

                    # software pipeline: emit S(kt) one step ahead of its
                    # exp/mask/PV consumers so PE never waits for ScalarE.
                    for kt in range(n_kt + 1):
                        if kt < n_kt:
                            emit_s(kt)
                        if kt >= 2:
                            flush_one()
                        if kt >= 1:
                            emit_consume(kt - 1)
                    emit_tail(p, q0, o_a, o_b)
                for tt in range(qc * 4, qc * 4 + 4):
                    for oc in range(2):
                        pending.append(make_proj_piece(tt, oc))
            flush_pending()
            if debug_taps:
                for p in range(PAIRS):
                    nc.sync.dma_start(dbg["y"][p * 128:(p + 1) * 128, :],
                                      y_sb[p])
                    nc.sync.dma_start(dbg["qt"][p * 128:(p + 1) * 128, :],
                                      qt_sb[p])
                    nc.sync.dma_start(dbg["kt"][p * 128:(p + 1) * 128, :],
                                      kt_sb[p])
                for h in range(NH_LOC):
                    nc.sync.dma_start(
                        dbg["v"][h * 128:(h + 1) * 128, :],
                        v_sb[h].rearrange("p a b -> p (a b)"))

    nc.compile()
    return nc


_NC_CACHE = {}


def _get_nc():
    if "nc" not in _NC_CACHE:
        _NC_CACHE["nc"] = build_nc()
    return _NC_CACHE["nc"]


def shard_inputs(x, W_attn, b_attn, W_proj):
    """Per-core input maps. Core c: batch c//2, head group c%2."""
    bf = ml_dtypes.bfloat16
    x = np.ascontiguousarray(np.asarray(x, dtype=np.float32))
    W_attn = np.asarray(W_attn, dtype=np.float32)
    b_attn = np.asarray(b_attn, dtype=np.float32)
    W_proj = np.asarray(W_proj, dtype=np.float32)
    in_maps = []
    for c in range(N_CORES):
        b, hg = c // 2, c % 2
        qs, ks, vs = hg * J, C + hg * J, 2 * C + hg * J
        wqk = np.ascontiguousarray(np.concatenate(
            [W_attn[:, qs:qs + J], W_attn[:, ks:ks + J]], axis=1)).astype(bf)
        wv = np.ascontiguousarray(W_attn[:, vs:vs + J]).astype(bf)
        bqk = np.ascontiguousarray(
            np.concatenate([b_attn[qs:qs + J], b_attn[ks:ks + J]]))
        bv = np.ascontiguousarray(b_attn[vs:vs + J]).astype(bf)
        wp = np.ascontiguousarray(W_proj[hg * J:(hg + 1) * J, :]).astype(bf)
        in_maps.append({
            "x": np.ascontiguousarray(x[b]),
            "wqk": wqk, "wv": wv, "bqk": bqk, "bv": bv, "wp": wp,
        })
    return in_maps


def kernel(x, W_attn, b_attn, W_proj, b_proj):
    nc = _get_nc()
    in_maps = shard_inputs(x, W_attn, b_attn, W_proj)
    res = run_bass_kernel_spmd(nc, in_maps, list(range(N_CORES)))
    b_proj = np.asarray(b_proj, dtype=np.float32)
    outs = []
    for b in range(4):
        partial = res.results[2 * b]["out"] + res.results[2 * b + 1]["out"]
        outs.append(partial + b_proj[None, :])
    return np.stack(outs, axis=0)


# revision 34
# speedup vs baseline: 2.6473x; 1.0399x over previous
"""Causal self-attention (B=4, T=2048, C=1024, 16 heads) on 8 trn2 NeuronCores.

Sharding: batch x head-group hybrid. Core c handles batch c//2 and head
group c%2 (8 of 16 heads). Each core computes the qkv projection for its
head group over its batch's tokens, runs causal attention for its 8
heads, and produces a partial c_proj output (contraction over its 512 of
the 1024 y channels). Host sums the two partials per batch, adds b_proj.

PE contracts over the partition dim, so x is laid out transposed (x^T)
once via PE transposes; after that every matmul chains without further
transposes:
  x^T [c, tok]        PE transpose (fp32 in, bf16 out)
  Q^T, K^T [j, tok]   = W_qk^T x^T   (j head-major, bf16)
  V' [tok, 65]        = x W_v        (bf16; col 64 = ones so that P@V'
                                      also emits softmax denominators)
  S^T [k_tok, q]      = K^T_tile.T Q^T  two heads packed per PE pass via
                        row groups (contract dim is 64); both heads land
                        in one 2-bank PSUM tile so a single ScalarE exp
                        covers them.
  P = exp(S^T/8)      bf16; causal diagonal blocks masked by a triu
                        multiply; fully-masked columns never computed.
  O' [65, q]          = V'.T P accumulated over k tiles.
  y [128, tok]        per head pair, bf16. Head B's O' rows are shifted
                        into partitions 64..128 by a SBUF->SBUF DMA
                        (compute engines cannot cross partitions).
  out partial [tok, C] = y_pair.T W_proj_rows accumulated over pairs.

Scheduling notes:
  - All matmul operands are bf16 (fp32 matmuls cost 4 cyc/row, bf16 1).
  - Phase A transposes for chunk qc+1 are emitted before the projection
    matmuls of chunk qc so PE is never starved by the PSUM->SBUF copies.
  - Attention loops qc-outer so the output projection of chunk qc can be
    emitted (and run) while attention for qc+1 proceeds.
  - Each group's normalization tail (copies -> SBUF DMAs -> reciprocal)
    runs on DVE/DMA in the background; the PE-touching finish (selector
    broadcast matmul + multiply) is deferred into the next group so
    neither PE nor VectorE stalls behind the chain. (GpSimd
    partition_broadcast looked ideal here but corrupts SBUF on real HW.)
Measured end-to-end relative error vs the fp32 reference: ~2e-3.
"""

from contextlib import ExitStack

import numpy as np
import ml_dtypes

import concourse.bass as bass
import concourse.mybir as mybir
import concourse.tile as tile
from concourse import bacc
from concourse.bass_utils import run_bass_kernel_spmd
from concourse.masks import make_identity

F32 = mybir.dt.float32
BF16 = mybir.dt.bfloat16

T = 2048
C = 1024
NH_LOC = 8          # heads per core
HD = 64
J = NH_LOC * HD     # 512 local q/k/v channels
N_CORES = 8
QC = 4              # q chunks of 512
TOK_TILES = 16      # token tiles of 128
C_TILES = 8         # contraction tiles of 128 over C
PAIRS = 4           # head pairs per core


def build_nc(debug_taps=False):
    nc = bacc.Bacc("TRN2", target_bir_lowering=False, debug=False)
    dbg = {}
    if debug_taps:
        dbg["y"] = nc.dram_tensor("dbg_y", [PAIRS * 128, T], BF16,
                                  kind="ExternalOutput")
        dbg["sums"] = nc.dram_tensor("dbg_sums", [PAIRS * QC * 2, 512], F32,
                                     kind="ExternalOutput")
        dbg["qt"] = nc.dram_tensor("dbg_qt", [PAIRS * 128, T], BF16,
                                   kind="ExternalOutput")
        dbg["kt"] = nc.dram_tensor("dbg_kt", [PAIRS * 128, T], BF16,
                                   kind="ExternalOutput")
        dbg["v"] = nc.dram_tensor("dbg_v", [NH_LOC * 128, TOK_TILES * 65],
                                  BF16, kind="ExternalOutput")

    x_d = nc.dram_tensor("x", [T, C], F32, kind="ExternalInput")
    wqk_d = nc.dram_tensor("wqk", [C, 2 * J], BF16, kind="ExternalInput")
    wv_d = nc.dram_tensor("wv", [C, J], BF16, kind="ExternalInput")
    bqk_d = nc.dram_tensor("bqk", [2 * J], F32, kind="ExternalInput")
    bv_d = nc.dram_tensor("bv", [J], BF16, kind="ExternalInput")
    wp_d = nc.dram_tensor("wp", [J, C], BF16, kind="ExternalInput")
    out_d = nc.dram_tensor("out", [T, C], F32, kind="ExternalOutput")

    with tile.TileContext(nc) as tc, ExitStack() as ctx:
        const = ctx.enter_context(tc.tile_pool(name="const", bufs=1))
        wpool = ctx.enter_context(tc.tile_pool(name="w", bufs=1))
        qkv = ctx.enter_context(tc.tile_pool(name="qkv", bufs=1))
        ypool = ctx.enter_context(tc.tile_pool(name="y", bufs=1))
        wk = ctx.enter_context(tc.tile_pool(name="wk", bufs=1))

        # ---- constants ----
        ident = const.tile([128, 128], F32)
        make_identity(nc, ident)
        # triu2[p, c, f] = 1 iff f >= p, duplicated over c: masks the causal
        # diagonal 128-block of both heads' P in one tensor_tensor op.
        triu2 = const.tile([128, 2, 128], BF16)
        nc.gpsimd.memset(triu2, 0.0)
        nc.gpsimd.affine_select(
            out=triu2, in_=triu2, compare_op=mybir.AluOpType.is_gt,
            fill=1.0, base=0, pattern=[[0, 2], [-1, 128]],
            channel_multiplier=1)
        ones_row = const.tile([1, 128], BF16)
        nc.vector.memset(ones_row, 1.0)
        # selab[p, f] = 1 iff f in [64p, 64p+64): head selector for the
        # reciprocal broadcast matmul (partition-1 memsets are illegal).
        selab = const.tile([2, 128], F32)
        nc.gpsimd.memset(selab, 1.0)
        nc.gpsimd.affine_select(
            out=selab, in_=selab, compare_op=mybir.AluOpType.is_ge,
            fill=0.0, base=0, pattern=[[1, 128]], channel_multiplier=-64)
        nc.gpsimd.affine_select(
            out=selab, in_=selab, compare_op=mybir.AluOpType.is_ge,
            fill=0.0, base=63, pattern=[[-1, 128]], channel_multiplier=64)
        bqk_sb = const.tile([128, 8], F32)
        nc.sync.dma_start(bqk_sb, bqk_d[:].rearrange("(t p) -> p t", p=128))
        bv_sb = const.tile([1, J], BF16)
        nc.sync.dma_start(bv_sb, bv_d[:].rearrange("(a n) -> a n", a=1))

        # ---- resident weights (bf16) ----
        wqk_sb = []
        for ct in range(C_TILES):
            w = wpool.tile([128, 2 * J], BF16, name=f"wqk{ct}")
            nc.sync.dma_start(w, wqk_d[ct * 128:(ct + 1) * 128, :])
            wqk_sb.append(w)
        wv_sb = []
        for ct in range(C_TILES):
            w = wpool.tile([128, J], BF16, name=f"wv{ct}")
            nc.sync.dma_start(w, wv_d[ct * 128:(ct + 1) * 128, :])
            wv_sb.append(w)
        wp_sb = []
        for p in range(PAIRS):
            w = wpool.tile([128, C], BF16, name=f"wp{p}")
            nc.sync.dma_start(w, wp_d[p * 128:(p + 1) * 128, :])
            wp_sb.append(w)

        # ---- persistent activations ----
        qt_sb = [qkv.tile([128, T], BF16, name=f"qt{p}") for p in range(PAIRS)]
        kt_sb = [qkv.tile([128, T], BF16, name=f"kt{p}") for p in range(PAIRS)]
        v_sb = [qkv.tile([128, TOK_TILES, 65], BF16, name=f"v{h}")
                for h in range(NH_LOC)]
        for h in range(NH_LOC):
            nc.vector.memset(v_sb[h][:, :, 64:65], 1.0)
        y_sb = [ypool.tile([128, T], BF16, name=f"y{p}") for p in range(PAIRS)]

        # ================= phase A: x^T, qkv projections =================
        with tc.tile_pool(name="psa", bufs=1, space="PSUM") as psa:

            def emit_transposes(qc):
                xT = wk.tile([128, C_TILES, 512], BF16, tag="xT", bufs=2)
                for tt in range(4):
                    xa = wk.tile([128, C], F32, tag="x", bufs=8)
                    r0 = qc * 512 + tt * 128
                    nc.sync.dma_start(xa, x_d[r0:r0 + 128, :])
                    for ct in range(C_TILES):
                        tp = psa.tile([128, 128], F32, tag="tp", bufs=4)
                        nc.tensor.transpose(
                            tp, xa[:, ct * 128:(ct + 1) * 128], ident)
                        nc.vector.tensor_copy(
                            xT[:, ct, tt * 128:(tt + 1) * 128], tp)
                return xT

            def emit_projs(qc, xT):
                # QK projection for this token chunk -> Q^T/K^T columns
                for jt in range(8):
                    pm = psa.tile([128, 512], F32, tag="mm", bufs=4)
                    for ct in range(C_TILES):
                        nc.tensor.matmul(
                            pm,
                            wqk_sb[ct][:, jt * 128:(jt + 1) * 128],
                            xT[:, ct, :],
                            start=(ct == 0), stop=(ct == C_TILES - 1))
                    dst = qt_sb[jt] if jt < 4 else kt_sb[jt - 4]
                    nc.scalar.activation(
                        dst[:, qc * 512:(qc + 1) * 512], pm,
                        mybir.ActivationFunctionType.Identity,
                        bias=bqk_sb[:, jt:jt + 1])
                # V projection for this token chunk (natural layout + bias)
                for tt in range(4):
                    tta = qc * 4 + tt
                    pv = psa.tile([128, J], F32, tag="mm", bufs=4)
                    for ct in range(C_TILES):
                        nc.tensor.matmul(
                            pv,
                            xT[:, ct, tt * 128:(tt + 1) * 128],
                            wv_sb[ct],
                            start=(ct == 0), stop=False)
                    nc.tensor.matmul(pv, ones_row, bv_sb,
                                     start=False, stop=True)
                    for h in range(NH_LOC):
                        # ScalarE is idle in phase A; keep VectorE for x^T
                        nc.scalar.copy(
                            v_sb[h][:, tta, 0:64], pv[:, h * 64:(h + 1) * 64])

            xTs = {0: emit_transposes(0)}
            for qc in range(QC):
                if qc + 1 < QC:
                    xTs[qc + 1] = emit_transposes(qc + 1)
                emit_projs(qc, xTs.pop(qc))

        # ============ phase B/D: attention + overlapped projection ========
        with tc.tile_pool(name="psb", bufs=1, space="PSUM") as psb:
            # Each group's normalization tail is a long cross-engine chain:
            # copies -> SBUF DMAs -> reciprocal -> partition-broadcast ->
            # multiply. Nothing in it runs on PE; the final multiply is
            # deferred into the next group so VectorE isn't blocked behind
            # the chain when the next group's mask multiplies come up.
            pending = []

            def flush_one():
                if pending:
                    pending.pop(0)()

            def flush_pending():
                while pending:
                    pending.pop(0)()

            def emit_tail(p, q0, o_a, o_b):
                # head A rows land aligned; stage sums + head B rows
                nc.vector.tensor_copy(y_sb[p][0:64, q0:q0 + 512],
                                      o_a[0:64, :])
                stg_b = wk.tile([64, 512], BF16, tag="stgb", bufs=2)
                nc.vector.tensor_copy(stg_b, o_b[0:64, :])
                stg_s = wk.tile([65, 1024], F32, tag="stgs", bufs=2)
                nc.vector.tensor_copy(stg_s[64:65, 0:512], o_a[64:65, :])
                nc.vector.tensor_copy(stg_s[64:65, 512:1024], o_b[64:65, :])
                sums = wk.tile([2, 512], F32, tag="sums", bufs=2)
                nc.sync.dma_start(sums[0:1, :], stg_s[64:65, 0:512])
                nc.sync.dma_start(sums[1:2, :], stg_s[64:65, 512:1024])
                nc.sync.dma_start(y_sb[p][64:128, q0:q0 + 512], stg_b)
                if debug_taps:
                    row = (p * QC + q0 // 512) * 2
                    nc.sync.dma_start(dbg["sums"][row:row + 2, :], sums)
                rec = wk.tile([2, 512], F32, tag="rec", bufs=2)
                nc.vector.reciprocal_approx_fast(rec, sums)

                def fin():
                    bc = psb.tile([128, 512], F32, tag="bcpo", bufs=1, name="bc")
                    nc.tensor.matmul(bc, selab, rec, start=True, stop=True)
                    nc.vector.tensor_mul(y_sb[p][:, q0:q0 + 512],
                                         y_sb[p][:, q0:q0 + 512], bc)
                pending.append(fin)

            def make_proj_piece(tt, oc):
                # one output-projection accumulation for token tile tt
                # (needs all 4 pairs' y columns for tt normalized)
                def proj():
                    po = psb.tile([128, 512], F32, tag="bcpo", bufs=1,
                                  name="po")
                    for p in range(PAIRS):
                        nc.tensor.matmul(
                            po,
                            y_sb[p][:, tt * 128:(tt + 1) * 128],
                            wp_sb[p][:, oc * 512:(oc + 1) * 512],
                            start=(p == 0), stop=(p == PAIRS - 1))
                    ob = wk.tile([128, 512], F32, tag="ob", bufs=4)
                    nc.vector.tensor_copy(ob, po)
                    nc.sync.dma_start(
                        out_d[tt * 128:(tt + 1) * 128,
                              oc * 512:(oc + 1) * 512], ob)
                return proj

            for qc in range(QC):
                q0 = qc * 512
                n_kt = 4 * (qc + 1)
                for p in range(PAIRS):
                    o_a = psb.tile([65, 512], F32, tag="O", bufs=3, name="o_a")
                    o_b = psb.tile([65, 512], F32, tag="O", bufs=3, name="o_b")
                    # software pipeline: emit S(kt) one step ahead of its
                    # exp/mask/PV consumers so PE never waits for ScalarE.
                    staged = {}

                    def emit_s(kt):
                        off = max(0, kt * 128 - q0)
                        # S for both heads in one 2-bank psum tile so one
                        # ScalarE exp covers both
                        s_ab = psb.tile([128, 1024], F32, tag="S", bufs=2,
                                        name="s_ab")
                        for half in range(2):
                            r0, r1 = half * 64, half * 64 + 64
                            nc.tensor.matmul(
                                s_ab[:, half * 512 + off:half * 512 + 512],
                                kt_sb[p][r0:r1, kt * 128:(kt + 1) * 128],
                                qt_sb[p][r0:r1, q0 + off:q0 + 512],
                                start=True, stop=True)
                        staged[kt] = (s_ab, off)

                    def emit_consume(kt):
                        s_ab, off = staged.pop(kt)
                        p_ab = wk.tile([128, 1024], BF16, tag="P", bufs=6,
                                       name="p_ab")
                        s3 = s_ab.rearrange("p (c w) -> p c w", c=2)
                        p3 = p_ab.rearrange("p (c w) -> p c w", c=2)
                        nc.scalar.activation(
                            p3[:, :, off:512], s3[:, :, off:512],
                            mybir.ActivationFunctionType.Exp, scale=0.125)
                        if kt * 128 >= q0:  # causal diagonal block
                            nc.vector.tensor_mul(
                                p3[:, :, off:off + 128],
                                p3[:, :, off:off + 128], triu2)
                        first, last = (kt == 0), (kt == n_kt - 1)
                        nc.tensor.matmul(o_a[:, off:512],
                                         v_sb[2 * p][:, kt, :],
                                         p_ab[:, off:512],
                                         start=first, stop=last)
                        nc.tensor.matmul(o_b[:, off:512],
                                         v_sb[2 * p + 1][:, kt, :],
                                         p_ab[:, 512 + off:1024],
                                         start=first, stop=last)

                    # software pipeline: emit S(kt) one step ahead of its
                    # exp/mask/PV consumers so PE never waits for ScalarE.
                    for kt in range(n_kt + 1):
                        if kt < n_kt:
                            emit_s(kt)
                        if kt >= 2:
                            flush_one()
                        if kt >= 1:
                            emit_consume(kt - 1)
                    emit_tail(p, q0, o_a, o_b)
                for tt in range(qc * 4, qc * 4 + 4):
                    for oc in range(2):
                        pending.append(make_proj_piece(tt, oc))
            flush_pending()
            if debug_taps:
                for p in range(PAIRS):
                    nc.sync.dma_start(dbg["y"][p * 128:(p + 1) * 128, :],
                                      y_sb[p])
                    nc.sync.dma_start(dbg["qt"][p * 128:(p + 1) * 128, :],
                                      qt_sb[p])
                    nc.sync.dma_start(dbg["kt"][p * 128:(p + 1) * 128, :],
                                      kt_sb[p])
                for h in range(NH_LOC):
                    nc.sync.dma_start(
                        dbg["v"][h * 128:(h + 1) * 128, :],
                        v_sb[h].rearrange("p a b -> p (a b)"))

    nc.compile()
    return nc


_NC_CACHE = {}


def _get_nc():
    if "nc" not in _NC_CACHE:
        _NC_CACHE["nc"] = build_nc()
    return _NC_CACHE["nc"]


def shard_inputs(x, W_attn, b_attn, W_proj):
    """Per-core input maps. Core c: batch c//2, head group c%2."""
    bf = ml_dtypes.bfloat16
    x = np.ascontiguousarray(np.asarray(x, dtype=np.float32))
    W_attn = np.asarray(W_attn, dtype=np.float32)
    b_attn = np.asarray(b_attn, dtype=np.float32)
    W_proj = np.asarray(W_proj, dtype=np.float32)
    in_maps = []
    for c in range(N_CORES):
        b, hg = c // 2, c % 2
        qs, ks, vs = hg * J, C + hg * J, 2 * C + hg * J
        wqk = np.ascontiguousarray(np.concatenate(
            [W_attn[:, qs:qs + J], W_attn[:, ks:ks + J]], axis=1)).astype(bf)
        wv = np.ascontiguousarray(W_attn[:, vs:vs + J]).astype(bf)
        bqk = np.ascontiguousarray(
            np.concatenate([b_attn[qs:qs + J], b_attn[ks:ks + J]]))
        bv = np.ascontiguousarray(b_attn[vs:vs + J]).astype(bf)
        wp = np.ascontiguousarray(W_proj[hg * J:(hg + 1) * J, :]).astype(bf)
        in_maps.append({
            "x": np.ascontiguousarray(x[b]),
            "wqk": wqk, "wv": wv, "bqk": bqk, "bv": bv, "wp": wp,
        })
    return in_maps


def kernel(x, W_attn, b_attn, W_proj, b_proj):
    nc = _get_nc()
    in_maps = shard_inputs(x, W_attn, b_attn, W_proj)
    res = run_bass_kernel_spmd(nc, in_maps, list(range(N_CORES)))
    b_proj = np.asarray(b_proj, dtype=np.float32)
    outs = []
    for b in range(4):
        partial = res.results[2 * b]["out"] + res.results[2 * b + 1]["out"]
        outs.append(partial + b_proj[None, :])
    return np.stack(outs, axis=0)


# revision 46
# speedup vs baseline: 2.7193x; 1.0272x over previous
"""Causal self-attention (B=4, T=2048, C=1024, 16 heads) on 8 trn2 NeuronCores.

Sharding: batch x head-group hybrid. Core c handles batch c//2 and head
group c%2 (8 of 16 heads). Each core computes the qkv projection for its
head group over its batch's tokens, runs causal attention for its 8
heads, and produces a partial c_proj output (contraction over its 512 of
the 1024 y channels). Host sums the two partials per batch, adds b_proj.

PE contracts over the partition dim, so x is laid out transposed (x^T)
once via PE transposes; after that every matmul chains without further
transposes:
  x^T [c, tok]        PE transpose (fp32 in, bf16 out)
  Q^T, K^T [j, tok]   = W_qk^T x^T   (j head-major, bf16)
  V' [tok, 65]        = x W_v        (bf16; col 64 = ones so that P@V'
                                      also emits softmax denominators)
  S^T [k_tok, q]      = K^T_tile.T Q^T  two heads packed per PE pass via
                        row groups (contract dim is 64); both heads land
                        in one 2-bank PSUM tile so a single ScalarE exp
                        covers them.
  P = exp(S^T/8)      bf16; causal diagonal blocks masked by a triu
                        multiply; fully-masked columns never computed.
  O' [65, q]          = V'.T P accumulated over k tiles.
  y [128, tok]        per head pair, bf16. Head B's O' rows are shifted
                        into partitions 64..128 by a SBUF->SBUF DMA
                        (compute engines cannot cross partitions).
  out partial [tok, C] = y_pair.T W_proj_rows accumulated over pairs.

Scheduling notes:
  - All matmul operands are bf16 (fp32 matmuls cost 4 cyc/row, bf16 1).
  - Phase A transposes for chunk qc+1 are emitted before the projection
    matmuls of chunk qc so PE is never starved by the PSUM->SBUF copies.
  - Attention loops qc-outer so the output projection of chunk qc can be
    emitted (and run) while attention for qc+1 proceeds.
  - Each group's normalization tail (copies -> SBUF DMAs -> reciprocal)
    runs on DVE/DMA in the background; the PE-touching finish (selector
    broadcast matmul + multiply) is deferred into the next group so
    neither PE nor VectorE stalls behind the chain. (GpSimd
    partition_broadcast looked ideal here but corrupts SBUF on real HW.)
Measured end-to-end relative error vs the fp32 reference: ~2e-3.
"""

from contextlib import ExitStack

import numpy as np
import ml_dtypes

import concourse.bass as bass
import concourse.mybir as mybir
import concourse.tile as tile
from concourse import bacc
from concourse.bass_utils import run_bass_kernel_spmd
from concourse.masks import make_identity

F32 = mybir.dt.float32
BF16 = mybir.dt.bfloat16

T = 2048
C = 1024
NH_LOC = 8          # heads per core
HD = 64
J = NH_LOC * HD     # 512 local q/k/v channels
N_CORES = 8
QC = 4              # q chunks of 512
TOK_TILES = 16      # token tiles of 128
C_TILES = 8         # contraction tiles of 128 over C
PAIRS = 4           # head pairs per core


def build_nc(debug_taps=False):
    nc = bacc.Bacc("TRN2", target_bir_lowering=False, debug=False)
    dbg = {}
    if debug_taps:
        dbg["y"] = nc.dram_tensor("dbg_y", [PAIRS * 128, T], BF16,
                                  kind="ExternalOutput")
        dbg["sums"] = nc.dram_tensor("dbg_sums", [PAIRS * QC * 2, 512], F32,
                                     kind="ExternalOutput")
        dbg["qt"] = nc.dram_tensor("dbg_qt", [PAIRS * 128, T], BF16,
                                   kind="ExternalOutput")
        dbg["kt"] = nc.dram_tensor("dbg_kt", [PAIRS * 128, T], BF16,
                                   kind="ExternalOutput")
        dbg["v"] = nc.dram_tensor("dbg_v", [NH_LOC * 128, TOK_TILES * 65],
                                  BF16, kind="ExternalOutput")

    x_d = nc.dram_tensor("x", [T, C], F32, kind="ExternalInput")
    wqk_d = nc.dram_tensor("wqk", [C, 2 * J], BF16, kind="ExternalInput")
    wv_d = nc.dram_tensor("wv", [C, J], BF16, kind="ExternalInput")
    bqk_d = nc.dram_tensor("bqk", [2 * J], F32, kind="ExternalInput")
    bv_d = nc.dram_tensor("bv", [J], BF16, kind="ExternalInput")
    wp_d = nc.dram_tensor("wp", [J, C], BF16, kind="ExternalInput")
    out_d = nc.dram_tensor("out", [T, C], F32, kind="ExternalOutput")

    with tile.TileContext(nc) as tc, ExitStack() as ctx:
        const = ctx.enter_context(tc.tile_pool(name="const", bufs=1))
        wpool = ctx.enter_context(tc.tile_pool(name="w", bufs=1))
        qkv = ctx.enter_context(tc.tile_pool(name="qkv", bufs=1))
        ypool = ctx.enter_context(tc.tile_pool(name="y", bufs=1))
        wk = ctx.enter_context(tc.tile_pool(name="wk", bufs=1))

        # ---- constants ----
        ident = const.tile([128, 128], F32)
        make_identity(nc, ident)
        # triu2[p, c, f] = 1 iff f >= p, duplicated over c: masks the causal
        # diagonal 128-block of both heads' P in one tensor_tensor op.
        triu2 = const.tile([128, 2, 128], BF16)
        nc.gpsimd.memset(triu2, 0.0)
        nc.gpsimd.affine_select(
            out=triu2, in_=triu2, compare_op=mybir.AluOpType.is_gt,
            fill=1.0, base=0, pattern=[[0, 2], [-1, 128]],
            channel_multiplier=1)
        ones_row = const.tile([1, 128], BF16)
        nc.vector.memset(ones_row, 1.0)
        # selab[p, f] = 1 iff f in [64p, 64p+64): head selector for the
        # reciprocal broadcast matmul (partition-1 memsets are illegal).
        selab = const.tile([2, 128], F32)
        nc.gpsimd.memset(selab, 1.0)
        nc.gpsimd.affine_select(
            out=selab, in_=selab, compare_op=mybir.AluOpType.is_ge,
            fill=0.0, base=0, pattern=[[1, 128]], channel_multiplier=-64)
        nc.gpsimd.affine_select(
            out=selab, in_=selab, compare_op=mybir.AluOpType.is_ge,
            fill=0.0, base=63, pattern=[[-1, 128]], channel_multiplier=64)
        bqk_sb = const.tile([128, 8], F32)
        nc.sync.dma_start(bqk_sb, bqk_d[:].rearrange("(t p) -> p t", p=128))
        bv_sb = const.tile([1, J], BF16)
        nc.sync.dma_start(bv_sb, bv_d[:].rearrange("(a n) -> a n", a=1))

        # ---- resident weights (bf16); DMAs emitted later, after chunk 0's
        # x loads, so the first transposes aren't queued behind 4MB ----
        wqk_sb = [wpool.tile([128, 2 * J], BF16, name=f"wqk{ct}")
                  for ct in range(C_TILES)]
        wv_sb = [wpool.tile([128, J], BF16, name=f"wv{ct}")
                 for ct in range(C_TILES)]
        wp_sb = [wpool.tile([128, C], BF16, name=f"wp{p}")
                 for p in range(PAIRS)]

        def emit_weight_dmas():
            for ct in range(C_TILES):
                nc.sync.dma_start(wqk_sb[ct], wqk_d[ct * 128:(ct + 1) * 128, :])
            for ct in range(C_TILES):
                nc.sync.dma_start(wv_sb[ct], wv_d[ct * 128:(ct + 1) * 128, :])
            for p in range(PAIRS):
                nc.sync.dma_start(wp_sb[p], wp_d[p * 128:(p + 1) * 128, :])

        # ---- persistent activations ----
        qt_sb = [qkv.tile([128, T], BF16, name=f"qt{p}") for p in range(PAIRS)]
        kt_sb = [qkv.tile([128, T], BF16, name=f"kt{p}") for p in range(PAIRS)]
        v_sb = [qkv.tile([128, TOK_TILES, 65], BF16, name=f"v{h}")
                for h in range(NH_LOC)]
        for h in range(NH_LOC):
            nc.vector.memset(v_sb[h][:, :, 64:65], 1.0)
        y_sb = [ypool.tile([128, T], BF16, name=f"y{p}") for p in range(PAIRS)]

        # ====== fused pipeline: qkv projection chunks overlap attention ====
        # One PSUM pool for the whole kernel (8 banks):
        #   S    [128,1024] x2  exp-pipeline score tiles            4 banks
        #   O    [65,512]   x2  O' accumulators (o_a, o_b)          2 banks
        #   acc  [128,512]  x1  qkv-projection + c_proj accums      1 bank
        #   bcpo [128,512]  x1  x^T transposes + recip broadcasts   1 bank
        # Phase A work for chunk qc+1 is sliced into pieces and drained
        # through the same deferred queue as the normalization finishes and
        # output-projection pieces, interleaving PE-heavy projection work
        # into the ScalarE-bound attention loop.
        with tc.tile_pool(name="ps", bufs=1, space="PSUM") as psb:
            pending = []

            def flush_one():
                if pending:
                    pending.pop(0)()

            def flush_pending():
                while pending:
                    pending.pop(0)()

            def a_pieces(qc, tr_tag="bcpo", tr_bufs=1, acc_tag="acc",
                         acc_bufs=1):
                """Emit-later closures computing x^T, Q^T/K^T, V' for qc.
                Chunk 0 runs before attention starts, so it may borrow the
                then-idle S/O banks for much deeper pipelining."""
                xT = wk.tile([128, C_TILES, 512], BF16, tag="xT", bufs=2,
                             name=f"xT{qc}")
                pieces = []

                def tr_piece(tt):
                    def run():
                        xa = wk.tile([128, C], F32, tag="x", bufs=4)
                        r0 = qc * 512 + tt * 128
                        nc.sync.dma_start(xa, x_d[r0:r0 + 128, :])
                        for ct in range(C_TILES):
                            tp = psb.tile([128, 128], F32, tag=tr_tag,
                                          bufs=tr_bufs, name="tp")
                            nc.tensor.transpose(
                                tp, xa[:, ct * 128:(ct + 1) * 128], ident)
                            nc.vector.tensor_copy(
                                xT[:, ct, tt * 128:(tt + 1) * 128], tp)
                    return run

                def qk_piece(jt):
                    def run():
                        pm = psb.tile([128, 512], F32, tag=acc_tag,
                                      bufs=acc_bufs, name="pm")
                        for ct in range(C_TILES):
                            nc.tensor.matmul(
                                pm,
                                wqk_sb[ct][:, jt * 128:(jt + 1) * 128],
                                xT[:, ct, :],
                                start=(ct == 0), stop=(ct == C_TILES - 1))
                        dst = qt_sb[jt] if jt < 4 else kt_sb[jt - 4]
                        nc.scalar.activation(
                            dst[:, qc * 512:(qc + 1) * 512], pm,
                            mybir.ActivationFunctionType.Identity,
                            bias=bqk_sb[:, jt:jt + 1])
                    return run

                def v_piece(tt):
                    def run():
                        tta = qc * 4 + tt
                        pv = psb.tile([128, J], F32, tag=acc_tag,
                                      bufs=acc_bufs, name="pv")
                        for ct in range(C_TILES):
                            nc.tensor.matmul(
                                pv,
                                xT[:, ct, tt * 128:(tt + 1) * 128],
                                wv_sb[ct],
                                start=(ct == 0), stop=False)
                        nc.tensor.matmul(pv, ones_row, bv_sb,
                                         start=False, stop=True)
                        for h in range(NH_LOC):
                            nc.scalar.copy(
                                v_sb[h][:, tta, 0:64],
                                pv[:, h * 64:(h + 1) * 64])
                    return run

                for tt in range(4):
                    pieces.append(tr_piece(tt))
                for jt in range(8):
                    pieces.append(qk_piece(jt))
                for tt in range(4):
                    pieces.append(v_piece(tt))
                return pieces

            def emit_tail(p, q0, o_a, o_b):
                # head A rows land aligned; stage sums + head B rows
                nc.vector.tensor_copy(y_sb[p][0:64, q0:q0 + 512],
                                      o_a[0:64, :])
                stg_b = wk.tile([64, 512], BF16, tag="stgb", bufs=2)
                nc.vector.tensor_copy(stg_b, o_b[0:64, :])
                stg_s = wk.tile([65, 1024], F32, tag="stgs", bufs=2)
                nc.vector.tensor_copy(stg_s[64:65, 0:512], o_a[64:65, :])
                nc.vector.tensor_copy(stg_s[64:65, 512:1024], o_b[64:65, :])
                sums = wk.tile([2, 512], F32, tag="sums", bufs=2)
                nc.sync.dma_start(sums[0:1, :], stg_s[64:65, 0:512])
                nc.sync.dma_start(sums[1:2, :], stg_s[64:65, 512:1024])
                nc.sync.dma_start(y_sb[p][64:128, q0:q0 + 512], stg_b)
                if debug_taps:
                    row = (p * QC + q0 // 512) * 2
                    nc.sync.dma_start(dbg["sums"][row:row + 2, :], sums)
                rec = wk.tile([2, 512], F32, tag="rec", bufs=2)
                nc.vector.reciprocal_approx_fast(rec, sums)

                def fin():
                    bc = psb.tile([128, 512], F32, tag="bcpo", bufs=1, name="bc")
                    nc.tensor.matmul(bc, selab, rec, start=True, stop=True)
                    nc.vector.tensor_mul(y_sb[p][:, q0:q0 + 512],
                                         y_sb[p][:, q0:q0 + 512], bc)
                pending.append(fin)

            def make_proj_piece(tt, oc, tag="bcpo", bufs=1):
                # one output-projection accumulation for token tile tt
                # (needs all 4 pairs' y columns for tt normalized)
                def proj():
                    po = psb.tile([128, 512], F32, tag=tag, bufs=bufs,
                                  name="po")
                    for p in range(PAIRS):
                        nc.tensor.matmul(
                            po,
                            y_sb[p][:, tt * 128:(tt + 1) * 128],
                            wp_sb[p][:, oc * 512:(oc + 1) * 512],
                            start=(p == 0), stop=(p == PAIRS - 1))
                    ob = wk.tile([128, 512], F32, tag="ob", bufs=4)
                    nc.vector.tensor_copy(ob, po)
                    nc.sync.dma_start(
                        out_d[tt * 128:(tt + 1) * 128,
                              oc * 512:(oc + 1) * 512], ob)
                return proj

            # chunk 0's projections run inline (nothing to overlap yet) on
            # the still-idle S/O banks; chunks 1..3 drain through the
            # pending queue during attention. Weight DMAs are emitted after
            # the first x loads (inside the transpose pieces) so the first
            # PE work isn't queued behind them.
            pcs0 = a_pieces(0, tr_tag="S", tr_bufs=2, acc_tag="O", acc_bufs=2)
            for piece in pcs0[:4]:
                piece()
            emit_weight_dmas()
            for piece in pcs0[4:]:
                piece()
            a_left = [0] * QC  # un-flushed A pieces per chunk

            def count_piece(piece, qc):
                def run():
                    a_left[qc] -= 1
                    piece()
                return run

            for qc in range(QC):
                q0 = qc * 512
                n_kt = 4 * (qc + 1)
                if qc + 1 < QC:
                    pcs = a_pieces(qc + 1)
                    a_left[qc + 1] = len(pcs)
                    pending.extend(count_piece(pc, qc + 1) for pc in pcs)
                # emission barrier: attention for qc depends on chunk qc's
                # Q/K/V writes being *emitted* (Tile tracks deps in trace
                # order); normally a no-op since pieces drain during qc-1.
                while a_left[qc] > 0:
                    flush_one()
                for p in range(PAIRS):
                    o_a = psb.tile([65, 512], F32, tag="O", bufs=2, name="o_a")
                    o_b = psb.tile([65, 512], F32, tag="O", bufs=2, name="o_b")
                    # software pipeline: emit S(kt) one step ahead of its
                    # exp/mask/PV consumers so PE never waits for ScalarE.
                    staged = {}

                    def emit_s(kt):
                        off = max(0, kt * 128 - q0)
                        # S for both heads in one 2-bank psum tile so one
                        # ScalarE exp covers both
                        s_ab = psb.tile([128, 1024], F32, tag="S", bufs=2,
                                        name="s_ab")
                        for half in range(2):
                            r0, r1 = half * 64, half * 64 + 64
                            nc.tensor.matmul(
                                s_ab[:, half * 512 + off:half * 512 + 512],
                                kt_sb[p][r0:r1, kt * 128:(kt + 1) * 128],
                                qt_sb[p][r0:r1, q0 + off:q0 + 512],
                                start=True, stop=True)
                        staged[kt] = (s_ab, off)

                    def emit_consume(kt):
                        s_ab, off = staged.pop(kt)
                        p_ab = wk.tile([128, 1024], BF16, tag="P", bufs=6,
                                       name="p_ab")
                        s3 = s_ab.rearrange("p (c w) -> p c w", c=2)
                        p3 = p_ab.rearrange("p (c w) -> p c w", c=2)
                        nc.scalar.activation(
                            p3[:, :, off:512], s3[:, :, off:512],
                            mybir.ActivationFunctionType.Exp, scale=0.125)
                        if kt * 128 >= q0:  # causal diagonal block
                            nc.vector.tensor_mul(
                                p3[:, :, off:off + 128],
                                p3[:, :, off:off + 128], triu2)
                        first, last = (kt == 0), (kt == n_kt - 1)
                        nc.tensor.matmul(o_a[:, off:512],
                                         v_sb[2 * p][:, kt, :],
                                         p_ab[:, off:512],
                                         start=first, stop=last)
                        nc.tensor.matmul(o_b[:, off:512],
                                         v_sb[2 * p + 1][:, kt, :],
                                         p_ab[:, 512 + off:1024],
                                         start=first, stop=last)

                    # software pipeline: emit S(kt) one step ahead of its
                    # exp/mask/PV consumers so PE never waits for ScalarE.
                    for kt in range(n_kt + 1):
                        if kt < n_kt:
                            emit_s(kt)
                        if kt >= 2:
                            flush_one()
                            flush_one()
                        if kt >= 1:
                            emit_consume(kt - 1)
                    emit_tail(p, q0, o_a, o_b)
                for tt in range(qc * 4, qc * 4 + 4):
                    for oc in range(2):
                        if qc == QC - 1:
                            # attention is over by the time these flush; the
                            # O banks are free for 2-deep pipelining
                            pending.append(make_proj_piece(tt, oc, "O", 2))
                        else:
                            pending.append(make_proj_piece(tt, oc))
            flush_pending()
            if debug_taps:
                for p in range(PAIRS):
                    nc.sync.dma_start(dbg["y"][p * 128:(p + 1) * 128, :],
                                      y_sb[p])
                    nc.sync.dma_start(dbg["qt"][p * 128:(p + 1) * 128, :],
                                      qt_sb[p])
                    nc.sync.dma_start(dbg["kt"][p * 128:(p + 1) * 128, :],
                                      kt_sb[p])
                for h in range(NH_LOC):
                    nc.sync.dma_start(
                        dbg["v"][h * 128:(h + 1) * 128, :],
                        v_sb[h].rearrange("p a b -> p (a b)"))

    nc.compile()
    return nc


_NC_CACHE = {}


def _get_nc():
    if "nc" not in _NC_CACHE:
        _NC_CACHE["nc"] = build_nc()
    return _NC_CACHE["nc"]


def shard_inputs(x, W_attn, b_attn, W_proj):
    """Per-core input maps. Core c: batch c//2, head group c%2."""
    bf = ml_dtypes.bfloat16
    x = np.ascontiguousarray(np.asarray(x, dtype=np.float32))
    W_attn = np.asarray(W_attn, dtype=np.float32)
    b_attn = np.asarray(b_attn, dtype=np.float32)
    W_proj = np.asarray(W_proj, dtype=np.float32)
    in_maps = []
    for c in range(N_CORES):
        b, hg = c // 2, c % 2
        qs, ks, vs = hg * J, C + hg * J, 2 * C + hg * J
        wqk = np.ascontiguousarray(np.concatenate(
            [W_attn[:, qs:qs + J], W_attn[:, ks:ks + J]], axis=1)).astype(bf)
        wv = np.ascontiguousarray(W_attn[:, vs:vs + J]).astype(bf)
        bqk = np.ascontiguousarray(
            np.concatenate([b_attn[qs:qs + J], b_attn[ks:ks + J]]))
        bv = np.ascontiguousarray(b_attn[vs:vs + J]).astype(bf)
        wp = np.ascontiguousarray(W_proj[hg * J:(hg + 1) * J, :]).astype(bf)
        in_maps.append({
            "x": np.ascontiguousarray(x[b]),
            "wqk": wqk, "wv": wv, "bqk": bqk, "bv": bv, "wp": wp,
        })
    return in_maps


def kernel(x, W_attn, b_attn, W_proj, b_proj):
    nc = _get_nc()
    in_maps = shard_inputs(x, W_attn, b_attn, W_proj)
    res = run_bass_kernel_spmd(nc, in_maps, list(range(N_CORES)))
    b_proj = np.asarray(b_proj, dtype=np.float32)
    outs = []
    for b in range(4):
        partial = res.results[2 * b]["out"] + res.results[2 * b + 1]["out"]
        outs.append(partial + b_proj[None, :])
    return np.stack(outs, axis=0)


# revision 49
# speedup vs baseline: 2.7766x; 1.0211x over previous
"""Causal self-attention (B=4, T=2048, C=1024, 16 heads) on 8 trn2 NeuronCores.

Sharding: batch x head-group hybrid. Core c handles batch c//2 and head
group c%2 (8 of 16 heads). Each core computes the qkv projection for its
head group over its batch's tokens, runs causal attention for its 8
heads, and produces a partial c_proj output (contraction over its 512 of
the 1024 y channels). Host sums the two partials per batch, adds b_proj.

PE contracts over the partition dim, so x is laid out transposed (x^T)
once via PE transposes; after that every matmul chains without further
transposes:
  x^T [c, tok]        PE transpose (fp32 in, bf16 out)
  Q^T, K^T [j, tok]   = W_qk^T x^T   (j head-major, bf16)
  V' [tok, 65]        = x W_v        (bf16; col 64 = ones so that P@V'
                                      also emits softmax denominators)
  S^T [k_tok, q]      = K^T_tile.T Q^T  two heads packed per PE pass via
                        row groups (contract dim is 64); both heads land
                        in one 2-bank PSUM tile so a single ScalarE exp
                        covers them.
  P = exp(S^T/8)      bf16; causal diagonal blocks masked by a triu
                        multiply; fully-masked columns never computed.
  O' [65, q]          = V'.T P accumulated over k tiles.
  y [128, tok]        per head pair, bf16. Head B's O' rows are shifted
                        into partitions 64..128 by a SBUF->SBUF DMA
                        (compute engines cannot cross partitions).
  out partial [tok, C] = y_pair.T W_proj_rows accumulated over pairs.

Scheduling notes:
  - All matmul operands are bf16 (fp32 matmuls cost 4 cyc/row, bf16 1).
  - Phase A transposes for chunk qc+1 are emitted before the projection
    matmuls of chunk qc so PE is never starved by the PSUM->SBUF copies.
  - Attention loops qc-outer so the output projection of chunk qc can be
    emitted (and run) while attention for qc+1 proceeds.
  - Each group's normalization tail (copies -> SBUF DMAs -> reciprocal)
    runs on DVE/DMA in the background; the PE-touching finish (selector
    broadcast matmul + multiply) is deferred into the next group so
    neither PE nor VectorE stalls behind the chain. (GpSimd
    partition_broadcast looked ideal here but corrupts SBUF on real HW.)
Measured end-to-end relative error vs the fp32 reference: ~2e-3.
"""

from contextlib import ExitStack

import numpy as np
import ml_dtypes

import concourse.bass as bass
import concourse.mybir as mybir
import concourse.tile as tile
from concourse import bacc
from concourse.bass_utils import run_bass_kernel_spmd
from concourse.masks import make_identity

F32 = mybir.dt.float32
BF16 = mybir.dt.bfloat16

T = 2048
C = 1024
NH_LOC = 8          # heads per core
HD = 64
J = NH_LOC * HD     # 512 local q/k/v channels
N_CORES = 8
QC = 4              # q chunks of 512
TOK_TILES = 16      # token tiles of 128
C_TILES = 8         # contraction tiles of 128 over C
PAIRS = 4           # head pairs per core


def build_nc(debug_taps=False):
    nc = bacc.Bacc("TRN2", target_bir_lowering=False, debug=False)
    dbg = {}
    if debug_taps:
        dbg["y"] = nc.dram_tensor("dbg_y", [PAIRS * 128, T], BF16,
                                  kind="ExternalOutput")
        dbg["sums"] = nc.dram_tensor("dbg_sums", [PAIRS * QC * 2, 512], F32,
                                     kind="ExternalOutput")
        dbg["qt"] = nc.dram_tensor("dbg_qt", [PAIRS * 128, T], BF16,
                                   kind="ExternalOutput")
        dbg["kt"] = nc.dram_tensor("dbg_kt", [PAIRS * 128, T], BF16,
                                   kind="ExternalOutput")
        dbg["v"] = nc.dram_tensor("dbg_v", [NH_LOC * 128, TOK_TILES * 65],
                                  BF16, kind="ExternalOutput")

    x_d = nc.dram_tensor("x", [T, C], F32, kind="ExternalInput")
    wqk_d = nc.dram_tensor("wqk", [C, 2 * J], BF16, kind="ExternalInput")
    wv_d = nc.dram_tensor("wv", [C, J], BF16, kind="ExternalInput")
    bqk_d = nc.dram_tensor("bqk", [2 * J], F32, kind="ExternalInput")
    bv_d = nc.dram_tensor("bv", [J], BF16, kind="ExternalInput")
    wp_d = nc.dram_tensor("wp", [J, C], BF16, kind="ExternalInput")
    out_d = nc.dram_tensor("out", [T, C], F32, kind="ExternalOutput")

    with tile.TileContext(nc) as tc, ExitStack() as ctx:
        const = ctx.enter_context(tc.tile_pool(name="const", bufs=1))
        wpool = ctx.enter_context(tc.tile_pool(name="w", bufs=1))
        qkv = ctx.enter_context(tc.tile_pool(name="qkv", bufs=1))
        ypool = ctx.enter_context(tc.tile_pool(name="y", bufs=1))
        wk = ctx.enter_context(tc.tile_pool(name="wk", bufs=1))

        # ---- constants ----
        ident = const.tile([128, 128], F32)
        make_identity(nc, ident)
        # triu2[p, c, f] = 1 iff f >= p, duplicated over c: masks the causal
        # diagonal 128-block of both heads' P in one tensor_tensor op.
        triu2 = const.tile([128, 2, 128], BF16)
        nc.gpsimd.memset(triu2, 0.0)
        nc.gpsimd.affine_select(
            out=triu2, in_=triu2, compare_op=mybir.AluOpType.is_gt,
            fill=1.0, base=0, pattern=[[0, 2], [-1, 128]],
            channel_multiplier=1)
        ones_row = const.tile([1, 128], BF16)
        nc.vector.memset(ones_row, 1.0)
        # selab[p, f] = 1 iff f in [64p, 64p+64): head selector for the
        # reciprocal broadcast matmul (partition-1 memsets are illegal).
        selab = const.tile([2, 128], F32)
        nc.gpsimd.memset(selab, 1.0)
        nc.gpsimd.affine_select(
            out=selab, in_=selab, compare_op=mybir.AluOpType.is_ge,
            fill=0.0, base=0, pattern=[[1, 128]], channel_multiplier=-64)
        nc.gpsimd.affine_select(
            out=selab, in_=selab, compare_op=mybir.AluOpType.is_ge,
            fill=0.0, base=63, pattern=[[-1, 128]], channel_multiplier=64)
        selab_r = const.tile([2, 128], mybir.dt.float32r)
        nc.vector.tensor_copy(selab_r, selab)
        bqk_sb = const.tile([128, 8], F32)
        nc.sync.dma_start(bqk_sb, bqk_d[:].rearrange("(t p) -> p t", p=128))
        bv_sb = const.tile([1, J], BF16)
        nc.sync.dma_start(bv_sb, bv_d[:].rearrange("(a n) -> a n", a=1))

        # ---- resident weights (bf16); DMAs emitted later, after chunk 0's
        # x loads, so the first transposes aren't queued behind 4MB ----
        wqk_sb = [wpool.tile([128, 2 * J], BF16, name=f"wqk{ct}")
                  for ct in range(C_TILES)]
        wv_sb = [wpool.tile([128, J], BF16, name=f"wv{ct}")
                 for ct in range(C_TILES)]
        wp_sb = [wpool.tile([128, C], BF16, name=f"wp{p}")
                 for p in range(PAIRS)]

        def emit_weight_dmas():
            for ct in range(C_TILES):
                nc.sync.dma_start(wqk_sb[ct], wqk_d[ct * 128:(ct + 1) * 128, :])
            for ct in range(C_TILES):
                nc.sync.dma_start(wv_sb[ct], wv_d[ct * 128:(ct + 1) * 128, :])
            for p in range(PAIRS):
                nc.sync.dma_start(wp_sb[p], wp_d[p * 128:(p + 1) * 128, :])

        # ---- persistent activations ----
        qt_sb = [qkv.tile([128, T], BF16, name=f"qt{p}") for p in range(PAIRS)]
        kt_sb = [qkv.tile([128, T], BF16, name=f"kt{p}") for p in range(PAIRS)]
        v_sb = [qkv.tile([128, TOK_TILES, 65], BF16, name=f"v{h}")
                for h in range(NH_LOC)]
        for h in range(NH_LOC):
            nc.vector.memset(v_sb[h][:, :, 64:65], 1.0)
        y_sb = [ypool.tile([128, T], BF16, name=f"y{p}") for p in range(PAIRS)]

        # ====== fused pipeline: qkv projection chunks overlap attention ====
        # One PSUM pool for the whole kernel (8 banks):
        #   S    [128,1024] x2  exp-pipeline score tiles            4 banks
        #   O    [65,512]   x2  O' accumulators (o_a, o_b)          2 banks
        #   acc  [128,512]  x1  qkv-projection + c_proj accums      1 bank
        #   bcpo [128,512]  x1  x^T transposes + recip broadcasts   1 bank
        # Phase A work for chunk qc+1 is sliced into pieces and drained
        # through the same deferred queue as the normalization finishes and
        # output-projection pieces, interleaving PE-heavy projection work
        # into the ScalarE-bound attention loop.
        with tc.tile_pool(name="ps", bufs=1, space="PSUM") as psb:
            pending = []

            def flush_one():
                if pending:
                    pending.pop(0)()

            def flush_pending():
                while pending:
                    pending.pop(0)()

            def a_pieces(qc, tr_tag="bcpo", tr_bufs=1, acc_tag="acc",
                         acc_bufs=1):
                """Emit-later closures computing x^T, Q^T/K^T, V' for qc.
                Chunk 0 runs before attention starts, so it may borrow the
                then-idle S/O banks for much deeper pipelining."""
                xT = wk.tile([128, C_TILES, 512], BF16, tag="xT", bufs=2,
                             name=f"xT{qc}")
                pieces = []

                def tr_piece(tt):
                    def run():
                        xa = wk.tile([128, C], F32, tag="x", bufs=4)
                        r0 = qc * 512 + tt * 128
                        nc.sync.dma_start(xa, x_d[r0:r0 + 128, :])
                        for ct in range(C_TILES):
                            tp = psb.tile([128, 128], F32, tag=tr_tag,
                                          bufs=tr_bufs, name="tp")
                            nc.tensor.transpose(
                                tp, xa[:, ct * 128:(ct + 1) * 128], ident)
                            nc.vector.tensor_copy(
                                xT[:, ct, tt * 128:(tt + 1) * 128], tp)
                    return run

                def qk_piece(jt):
                    def run():
                        pm = psb.tile([128, 512], F32, tag=acc_tag,
                                      bufs=acc_bufs, name="pm")
                        for ct in range(C_TILES):
                            nc.tensor.matmul(
                                pm,
                                wqk_sb[ct][:, jt * 128:(jt + 1) * 128],
                                xT[:, ct, :],
                                start=(ct == 0), stop=(ct == C_TILES - 1))
                        dst = qt_sb[jt] if jt < 4 else kt_sb[jt - 4]
                        nc.scalar.activation(
                            dst[:, qc * 512:(qc + 1) * 512], pm,
                            mybir.ActivationFunctionType.Identity,
                            bias=bqk_sb[:, jt:jt + 1])
                    return run

                def v_piece(tt):
                    def run():
                        tta = qc * 4 + tt
                        pv = psb.tile([128, J], F32, tag=acc_tag,
                                      bufs=acc_bufs, name="pv")
                        for ct in range(C_TILES):
                            nc.tensor.matmul(
                                pv,
                                xT[:, ct, tt * 128:(tt + 1) * 128],
                                wv_sb[ct],
                                start=(ct == 0), stop=False)
                        nc.tensor.matmul(pv, ones_row, bv_sb,
                                         start=False, stop=True)
                        for h in range(NH_LOC):
                            nc.scalar.copy(
                                v_sb[h][:, tta, 0:64],
                                pv[:, h * 64:(h + 1) * 64])
                    return run

                for tt in range(4):
                    pieces.append(tr_piece(tt))
                for jt in range(8):
                    pieces.append(qk_piece(jt))
                for tt in range(4):
                    pieces.append(v_piece(tt))
                return pieces

            def emit_tail(p, q0, o_a, o_b):
                # head A rows land aligned; stage sums + head B rows
                nc.vector.tensor_copy(y_sb[p][0:64, q0:q0 + 512],
                                      o_a[0:64, :])
                stg_b = wk.tile([64, 512], BF16, tag="stgb", bufs=2)
                nc.vector.tensor_copy(stg_b, o_b[0:64, :])
                stg_s = wk.tile([65, 1024], F32, tag="stgs", bufs=2)
                nc.vector.tensor_copy(stg_s[64:65, 0:512], o_a[64:65, :])
                nc.vector.tensor_copy(stg_s[64:65, 512:1024], o_b[64:65, :])
                sums = wk.tile([2, 512], F32, tag="sums", bufs=2)
                nc.sync.dma_start(sums[0:1, :], stg_s[64:65, 0:512])
                nc.sync.dma_start(sums[1:2, :], stg_s[64:65, 512:1024])
                nc.sync.dma_start(y_sb[p][64:128, q0:q0 + 512], stg_b)
                if debug_taps:
                    row = (p * QC + q0 // 512) * 2
                    nc.sync.dma_start(dbg["sums"][row:row + 2, :], sums)
                rec = wk.tile([2, 512], F32, tag="rec", bufs=2)
                nc.vector.reciprocal_approx_fast(rec, sums)
                # f32r matmul inputs must come from a rounding producer
                rec_r = wk.tile([2, 512], mybir.dt.float32r, tag="recr",
                                bufs=2)
                nc.vector.tensor_copy(rec_r, rec)

                def fin():
                    # f32r runs 1 cyc/row vs fp32's 4 (values are exact 0/1
                    # selector rows times fp32 reciprocals; f32r's reduced
                    # multiply precision is irrelevant here)
                    bc = psb.tile([128, 512], F32, tag="bcpo", bufs=1, name="bc")
                    nc.tensor.matmul(bc, selab_r, rec_r,
                                     start=True, stop=True)
                    nc.vector.tensor_mul(y_sb[p][:, q0:q0 + 512],
                                         y_sb[p][:, q0:q0 + 512], bc)
                pending.append(fin)

            def make_proj_piece(tt, oc, tag="bcpo", bufs=1):
                # one output-projection accumulation for token tile tt
                # (needs all 4 pairs' y columns for tt normalized)
                def proj():
                    po = psb.tile([128, 512], F32, tag=tag, bufs=bufs,
                                  name="po")
                    for p in range(PAIRS):
                        nc.tensor.matmul(
                            po,
                            y_sb[p][:, tt * 128:(tt + 1) * 128],
                            wp_sb[p][:, oc * 512:(oc + 1) * 512],
                            start=(p == 0), stop=(p == PAIRS - 1))
                    ob = wk.tile([128, 512], F32, tag="ob", bufs=4)
                    nc.vector.tensor_copy(ob, po)
                    nc.sync.dma_start(
                        out_d[tt * 128:(tt + 1) * 128,
                              oc * 512:(oc + 1) * 512], ob)
                return proj

            # chunk 0's projections run inline (nothing to overlap yet) on
            # the still-idle S/O banks; chunks 1..3 drain through the
            # pending queue during attention. Weight DMAs are emitted after
            # the first x loads (inside the transpose pieces) so the first
            # PE work isn't queued behind them.
            pcs0 = a_pieces(0, tr_tag="S", tr_bufs=2, acc_tag="O", acc_bufs=2)
            for piece in pcs0[:4]:
                piece()
            emit_weight_dmas()
            for piece in pcs0[4:]:
                piece()
            a_left = [0] * QC  # un-flushed A pieces per chunk

            def count_piece(piece, qc):
                def run():
                    a_left[qc] -= 1
                    piece()
                return run

            for qc in range(QC):
                q0 = qc * 512
                n_kt = 4 * (qc + 1)
                if qc + 1 < QC:
                    pcs = a_pieces(qc + 1)
                    a_left[qc + 1] = len(pcs)
                    pending.extend(count_piece(pc, qc + 1) for pc in pcs)
                # emission barrier: attention for qc depends on chunk qc's
                # Q/K/V writes being *emitted* (Tile tracks deps in trace
                # order); normally a no-op since pieces drain during qc-1.
                while a_left[qc] > 0:
                    flush_one()
                for p in range(PAIRS):
                    o_a = psb.tile([65, 512], F32, tag="O", bufs=2, name="o_a")
                    o_b = psb.tile([65, 512], F32, tag="O", bufs=2, name="o_b")
                    # software pipeline: emit S(kt) one step ahead of its
                    # exp/mask/PV consumers so PE never waits for ScalarE.
                    staged = {}

                    def emit_s(kt):
                        off = max(0, kt * 128 - q0)
                        # S for both heads in one 2-bank psum tile so one
                        # ScalarE exp covers both
                        s_ab = psb.tile([128, 1024], F32, tag="S", bufs=2,
                                        name="s_ab")
                        for half in range(2):
                            r0, r1 = half * 64, half * 64 + 64
                            nc.tensor.matmul(
                                s_ab[:, half * 512 + off:half * 512 + 512],
                                kt_sb[p][r0:r1, kt * 128:(kt + 1) * 128],
                                qt_sb[p][r0:r1, q0 + off:q0 + 512],
                                start=True, stop=True)
                        staged[kt] = (s_ab, off)

                    def emit_consume(kt):
                        s_ab, off = staged.pop(kt)
                        p_ab = wk.tile([128, 1024], BF16, tag="P", bufs=6,
                                       name="p_ab")
                        s3 = s_ab.rearrange("p (c w) -> p c w", c=2)
                        p3 = p_ab.rearrange("p (c w) -> p c w", c=2)
                        nc.scalar.activation(
                            p3[:, :, off:512], s3[:, :, off:512],
                            mybir.ActivationFunctionType.Exp, scale=0.125)
                        if kt * 128 >= q0:  # causal diagonal block
                            nc.vector.tensor_mul(
                                p3[:, :, off:off + 128],
                                p3[:, :, off:off + 128], triu2)
                        first, last = (kt == 0), (kt == n_kt - 1)
                        nc.tensor.matmul(o_a[:, off:512],
                                         v_sb[2 * p][:, kt, :],
                                         p_ab[:, off:512],
                                         start=first, stop=last)
                        nc.tensor.matmul(o_b[:, off:512],
                                         v_sb[2 * p + 1][:, kt, :],
                                         p_ab[:, 512 + off:1024],
                                         start=first, stop=last)

                    # software pipeline: emit S(kt) one step ahead of its
                    # exp/mask/PV consumers so PE never waits for ScalarE.
                    for kt in range(n_kt + 1):
                        if kt < n_kt:
                            emit_s(kt)
                        if kt >= 2:
                            flush_one()
                            flush_one()
                        if kt >= 1:
                            emit_consume(kt - 1)
                    emit_tail(p, q0, o_a, o_b)
                for tt in range(qc * 4, qc * 4 + 4):
                    for oc in range(2):
                        if qc == QC - 1:
                            # attention is over by the time these flush; the
                            # O banks are free for 2-deep pipelining
                            pending.append(make_proj_piece(tt, oc, "O", 2))
                        else:
                            pending.append(make_proj_piece(tt, oc))
            flush_pending()
            if debug_taps:
                for p in range(PAIRS):
                    nc.sync.dma_start(dbg["y"][p * 128:(p + 1) * 128, :],
                                      y_sb[p])
                    nc.sync.dma_start(dbg["qt"][p * 128:(p + 1) * 128, :],
                                      qt_sb[p])
                    nc.sync.dma_start(dbg["kt"][p * 128:(p + 1) * 128, :],
                                      kt_sb[p])
                for h in range(NH_LOC):
                    nc.sync.dma_start(
                        dbg["v"][h * 128:(h + 1) * 128, :],
                        v_sb[h].rearrange("p a b -> p (a b)"))

    nc.compile()
    return nc


_NC_CACHE = {}


def _get_nc():
    if "nc" not in _NC_CACHE:
        _NC_CACHE["nc"] = build_nc()
    return _NC_CACHE["nc"]


def shard_inputs(x, W_attn, b_attn, W_proj):
    """Per-core input maps. Core c: batch c//2, head group c%2."""
    bf = ml_dtypes.bfloat16
    x = np.ascontiguousarray(np.asarray(x, dtype=np.float32))
    W_attn = np.asarray(W_attn, dtype=np.float32)
    b_attn = np.asarray(b_attn, dtype=np.float32)
    W_proj = np.asarray(W_proj, dtype=np.float32)
    in_maps = []
    for c in range(N_CORES):
        b, hg = c // 2, c % 2
        qs, ks, vs = hg * J, C + hg * J, 2 * C + hg * J
        wqk = np.ascontiguousarray(np.concatenate(
            [W_attn[:, qs:qs + J], W_attn[:, ks:ks + J]], axis=1)).astype(bf)
        wv = np.ascontiguousarray(W_attn[:, vs:vs + J]).astype(bf)
        bqk = np.ascontiguousarray(
            np.concatenate([b_attn[qs:qs + J], b_attn[ks:ks + J]]))
        bv = np.ascontiguousarray(b_attn[vs:vs + J]).astype(bf)
        wp = np.ascontiguousarray(W_proj[hg * J:(hg + 1) * J, :]).astype(bf)
        in_maps.append({
            "x": np.ascontiguousarray(x[b]),
            "wqk": wqk, "wv": wv, "bqk": bqk, "bv": bv, "wp": wp,
        })
    return in_maps


def kernel(x, W_attn, b_attn, W_proj, b_proj):
    nc = _get_nc()
    in_maps = shard_inputs(x, W_attn, b_attn, W_proj)
    res = run_bass_kernel_spmd(nc, in_maps, list(range(N_CORES)))
    b_proj = np.asarray(b_proj, dtype=np.float32)
    outs = []
    for b in range(4):
        partial = res.results[2 * b]["out"] + res.results[2 * b + 1]["out"]
        outs.append(partial + b_proj[None, :])
    return np.stack(outs, axis=0)
